# revision 20
# baseline (speedup 1.0000x reference)
"""Cross-attention kernel for Trainium2 (8 NeuronCores, data-parallel over batch).

Per core (one batch b):
  q = Wq @ x; k = Wk @ xs; v = Wv @ xs          (channel mix, c=64 contraction)
  per head d:  S^T[g,h] = k_d q_d^T             (contract w)
               P^T = exp(S^T/8 + BIAS)          (no-max softmax; bias keeps fp16 exp in range)
               O[h,w] = P^T.T @ V_d ; Z[h] = P^T.T @ 1 ; out = O / Z

All device compute is fp16 (inputs host-cast; PSUM accumulates f32).

KV projection: weights stationary, row-packed chunk pairs (A on PE rows 0-63,
B on 64-127).  Each chunk's [128ch x 512] PSUM is drained by ONE full-width
128-partition copy into a combined ring (k on partitions 0-63, v on 64-127)
holding [128, 16 g-rows, 256 w].  Per 16-row block: one xbar transpose of the
k half -> Kt[w, blk, slot, whalf, c], one DMA of the v half -> HBM natural
[c, h, w] (re-loaded per head as [g, w] tiles).

Q projection: col-tiled pairs (tile_position (0,0)/(64,64)) put chunk A on
PSUM partitions 0-63, B on 64-127; x is loaded h-parity-split (even rows ->
low partitions) so the ring slot order is h-affine: h = 32*blk + 2*s + par.
One full-width copy drains a whole pair.  Per 32-row block one xbar ->
Qt[w, blk, s, whalf, par, c]; a per-head 3D repack (strided read, contiguous
write) gives qc[i][w, h] so the S-matmul rhs streams contiguously.

Attention per head: S/O matmuls rotate through a 6-buffer PSUM pool so heads
pipeline; exp on ACT; softmax normalization folded into a per-partition scalar
multiply; fp16 output, upcast on host.
"""

import sys

try:
    import concourse  # noqa: F401
except ImportError:  # pragma: no cover
    sys.path.insert(0, "/opt/trn_rl_repo")

import numpy as np

import concourse.bass as bass  # noqa: F401
from concourse import bacc
import concourse.mybir as mybir
import concourse.tile as tile

F32 = mybir.dt.float32
F16 = mybir.dt.float16

B = 8
C = 64
H = 256
W = 256
W2 = W // 2

TEMP_INV = 1.0 / float(np.sqrt(C))
EXP_BIAS = -5.0

CH = 512                      # positions per matmul chunk (= 2 spatial rows)
NPAIR = H * W // (2 * CH)     # 64 packed pairs per projection


def build_program():
    nc = bacc.Bacc("TRN2", target_bir_lowering=False, debug=False)

    x = nc.dram_tensor("x", [C, H, W], F16, kind="ExternalInput")
    xs = nc.dram_tensor("xs", [C, H, W], F16, kind="ExternalInput")
    wqT = nc.dram_tensor("wqT", [C, C], F16, kind="ExternalInput")
    wkvT = nc.dram_tensor("wkvT", [C, 2 * C], F16, kind="ExternalInput")
    out = nc.dram_tensor("out", [C, H, W], F16, kind="ExternalOutput")
    v_dram = nc.dram_tensor("v_dram", [C, H, W], F16, kind="Internal")

    xs_flat = xs.rearrange("c h w -> c (h w)")
    # x parity view: h = 16*U + 2*m + par  (U: 4-pair group, m: 8 even slots)
    x_par = x.rearrange("c (u m par) w -> c u par m w", m=8, par=2)

    with tile.TileContext(nc) as tc:
        with (
            tc.tile_pool(name="consts", bufs=1) as consts,
            tc.tile_pool(name="stage", bufs=1) as stage,
            tc.tile_pool(name="ins", bufs=3) as ins_pool,
            tc.tile_pool(name="inx", bufs=3) as inx_pool,
            tc.tile_pool(name="kvring", bufs=2) as kvring,
            tc.tile_pool(name="qring", bufs=2) as qring,
            tc.tile_pool(name="attn", bufs=4) as attn,
            tc.tile_pool(name="ps", bufs=5, space="PSUM") as ps_pool,
            tc.tile_pool(name="ps_q", bufs=3, space="PSUM") as psq_pool,
        ):
            # ---- constants ----
            wq2 = consts.tile([128, C], F16)
            wkv2 = consts.tile([128, 2 * C], F16)
            for hlf in range(2):
                nc.gpsimd.dma_start(wq2[hlf * C:(hlf + 1) * C, :], wqT[:])
                nc.gpsimd.dma_start(wkv2[hlf * C:(hlf + 1) * C, :], wkvT[:])
            bias_sb = consts.tile([128, 1], F32)
            nc.vector.memset(bias_sb[:], EXP_BIAS)

            # persistent V tiles (double-buffered): [g0 v | 1 | g1 v | 1]
            vhp = [consts.tile([128, 2 * (W + 1)], F16, name=f"vhp{i}") for i in range(2)]
            for i in range(2):
                nc.gpsimd.memset(vhp[i][:, W:W + 1], 1.0)
                nc.gpsimd.memset(vhp[i][:, 2 * W + 1:2 * W + 2], 1.0)

            # ---- persistent staging (fp16) ----
            # Kt: [w2, blk16, slot16, wh2, c64]   g = 16*blk + slot
            Kt = stage.tile([W2, H * 2 * C], F16, name="Kt")
            Kt_x = Kt.rearrange("w (b r c) -> w b r c", r=32, c=C)       # xbar view
            Kt_s = Kt.rearrange("w (b s wh c) -> w b s wh c", s=16, wh=2, c=C)
            # Qt: [w2, blk8, s16, wh2, par2, c64]  h = 32*blk + 2*s + par
            Qt = stage.tile([W2, H * 2 * C], F16, name="Qt")
            Qt_x = Qt.rearrange("w (b r c) -> w b r c", r=32, c=128)     # xbar view
            Qt_s = Qt.rearrange("w (b s wh par c) -> w b s wh par c", s=16, wh=2, par=2, c=C)

            # =================== KV projection ===================
            kv_state = {}

            def kv_drain(cidx, psb, eng):
                """chunk cidx covers g rows {2c, 2c+1}; one full-width copy."""
                sub = cidx % 8
                if sub == 0 and eng == 0:
                    kv_state["r"] = kvring.tile([128, 16, W], F16, tag="kvr", name="kvr")
                ring = kv_state["r"]
                dst = ring[:, sub * 2:sub * 2 + 2, :]
                src = psb.rearrange("p (g w) -> p g w", w=W)
                if eng == 0:
                    nc.scalar.copy(out=dst, in_=src)
                else:
                    nc.vector.tensor_copy(out=dst, in_=src)

            def kv_flush(blk):
                ring = kv_state["r"]
                gb0 = blk * 16
                nc.sync.dma_start_transpose(
                    out=Kt_x[:, blk, :, :], in_=ring[0:C, :, :]
                )
                nc.scalar.dma_start(
                    out=v_dram[:, gb0:gb0 + 16, :], in_=ring[C:128, :, :]
                )

            # =================== Q projection helpers ===================
            q_state = {}

            def q_drain(p, psq, eng):
                """pair p: one full-width copy; slots s=2*(p%8)+r,
                h = 32*blk + 2*s + par (par = partition half)."""
                pp = p % 8
                if pp == 0 and eng == 0:
                    q_state["r"] = qring.tile([128, 16, W], F16, tag="qr", name="qr")
                ring = q_state["r"]
                dst = ring[:, pp * 2:pp * 2 + 2, :]
                src = psq.rearrange("p (r w) -> p r w", w=W)
                if eng == 0:
                    nc.scalar.copy(out=dst, in_=src)
                else:
                    nc.vector.tensor_copy(out=dst, in_=src)

            def q_flush(blk):
                ring = q_state["r"]
                nc.sync.dma_start_transpose(out=Qt_x[:, blk, :, :], in_=ring[:])

            # PE warmup: ~9us of dense matmuls so projections start at K=8/8
            wtile = consts.tile([C, CH], F16, name="wtile")
            nc.vector.memset(wtile[:], 0.01)
            warm_ps = ps_pool.tile([128, CH], F32, tag="ps", name="warm")
            for _ in range(30):
                nc.tensor.matmul(
                    warm_ps[0:C, :], wtile[:, 0:C], wtile[:], start=True, stop=True
                )

            # ============ merged projection pass (KV + Q interleaved) ============
            for U in range(NPAIR // 4):
                sin4 = ins_pool.tile([128, 4 * CH], F16, tag="sin", name="sin4")
                base = U * 8 * CH
                nc.gpsimd.dma_start(sin4[0:C, :], xs_flat[:, base:base + 4 * CH])
                nc.gpsimd.dma_start(sin4[C:128, :], xs_flat[:, base + 4 * CH:base + 8 * CH])
                xin4 = inx_pool.tile([128, 4 * CH], F16, tag="xin", name="xin4")
                for par in range(2):
                    eng = nc.scalar if par == 0 else nc.gpsimd
                    eng.dma_start(
                        xin4[par * C:(par + 1) * C, :], x_par[:, U, par, :, :]
                    )
                for j in range(4):
                    ps_a = ps_pool.tile([128, CH], F32, tag="ps", name="kva")
                    ps_b = ps_pool.tile([128, CH], F32, tag="ps", name="kvb")
                    nc.tensor.matmul(
                        ps_a[:], wkv2[0:C, :], sin4[0:C, j * CH:(j + 1) * CH],
                        start=True, stop=True,
                    )
                    nc.tensor.matmul(
                        ps_b[:], wkv2[C:128, :], sin4[C:128, j * CH:(j + 1) * CH],
                        start=True, stop=True,
                    )
                    ca, cb = 8 * U + j, 8 * U + 4 + j
                    kv_drain(ca, ps_a, ca % 2)
                    kv_drain(cb, ps_b, cb % 2)
                    p = 4 * U + j
                    psq = psq_pool.tile([128, CH], F32, tag="q", name="psq")
                    nc.tensor.matmul(
                        psq[0:C, :], wq2[0:C, :], xin4[0:C, j * CH:(j + 1) * CH],
                        start=True, stop=True, tile_position=(0, 0),
                    )
                    nc.tensor.matmul(
                        psq[C:128, :], wq2[C:128, :], xin4[C:128, j * CH:(j + 1) * CH],
                        start=True, stop=True, tile_position=(64, 64),
                    )
                    q_drain(p, psq, p % 2)
                    if p % 8 == 7:
                        q_flush(p // 8)
                # chunks 8U..8U+7 fill the ring for g rows [16U, 16U+16)
                kv_flush(U)

            # =================== attention ===================
            def v_load(d):
                dst = vhp[d % 2].rearrange("p (g xx) -> p g xx", xx=W + 1)[:, :, 0:W]
                src = v_dram[d, :, :].rearrange("(g p) w -> p g w", g=2)
                nc.sync.dma_start(out=dst, in_=src)

            def q_repack(d):
                qc = [
                    attn.tile([W2, H], F16, tag=f"qc{i}", name=f"qc{i}")
                    for i in range(2)
                ]
                for i in range(2):
                    dst = qc[i].rearrange("w (b s par) -> w b s par", s=16, par=2)
                    src = Qt_s[:, :, :, i, :, d]
                    if i == 0:
                        nc.gpsimd.tensor_copy(out=dst, in_=src)
                    else:
                        nc.scalar.copy(out=dst, in_=src)
                return qc

            qc_cur = q_repack(0)
            v_load(0)
            for d in range(C):
                if d + 1 < C:
                    qc_nxt = q_repack(d + 1)
                    v_load(d + 1)
                else:
                    qc_nxt = None

                expS = []
                for gt in range(2):
                    st = ps_pool.tile([128, CH], F32, tag="ps", name="st")
                    for i in range(2):
                        nc.tensor.matmul(
                            st[:, 0:H],
                            Kt_s[:, 8 * gt:8 * gt + 8, :, i, d],
                            qc_cur[i][:],
                            start=(i == 0), stop=(i == 1),
                        )
                    e = attn.tile([128, H], F16, tag="expS", name="expS")
                    nc.scalar.activation(
                        out=e[:], in_=st[:, 0:H],
                        func=mybir.ActivationFunctionType.Exp,
                        bias=bias_sb[:], scale=TEMP_INV,
                    )
                    expS.append(e)

                vhp_v = vhp[d % 2].rearrange("p (g xx) -> p g xx", xx=W + 1)
                osb = attn.tile([128, 2 * W], F16, tag="osb", name="osb")
                r = attn.tile([128, 2], F32, tag="r", name="r")
                for hc in range(2):
                    ops = psq_pool.tile([128, CH], F32, tag="q", name="ops")
                    for gt in range(2):
                        nc.tensor.matmul(
                            ops[:, 0:W + 1], expS[gt][:, hc * 128:(hc + 1) * 128],
                            vhp_v[:, gt, :],
                            start=(gt == 0), stop=(gt == 1),
                        )
                    nc.vector.reciprocal(r[:, hc:hc + 1], ops[:, W:W + 1])
                    nc.vector.tensor_scalar_mul(
                        osb[:, hc * W:(hc + 1) * W], ops[:, 0:W], r[:, hc:hc + 1]
                    )
                out_v = out[d, :, :].rearrange("(hc p) w -> p hc w", hc=2)
                nc.sync.dma_start(out=out_v[:, 0, :], in_=osb[:, 0:W])
                nc.gpsimd.dma_start(out=out_v[:, 1, :], in_=osb[:, W:2 * W])

                qc_cur = qc_nxt



    nc.compile()
    return nc


_NC_CACHE = None


def _get_program():
    global _NC_CACHE
    if _NC_CACHE is None:
        _NC_CACHE = build_program()
    return _NC_CACHE


def kernel(x, x_s, Wq, Wkv):
    from concourse.bass_utils import run_bass_kernel_spmd

    nc = _get_program()
    wqT = np.ascontiguousarray(Wq.T).astype(np.float16)
    wkvT = np.ascontiguousarray(Wkv.T).astype(np.float16)
    in_maps = [
        {
            "x": np.ascontiguousarray(x[b]).astype(np.float16),
            "xs": np.ascontiguousarray(x_s[b]).astype(np.float16),
            "wqT": wqT,
            "wkvT": wkvT,
        }
        for b in range(B)
    ]
    res = run_bass_kernel_spmd(nc, in_maps, list(range(B)))
    return np.stack(
        [res.results[i]["out"].astype(np.float32) for i in range(B)], axis=0
    )
